# revision 12
# baseline (speedup 1.0000x reference)
"""Trainium2 Bass kernel for the LSTM decoder problem (nn_Decoder).

Math (reference):
    h0 = latent @ W_fc.T + b_fc ;  c0 = 0 ;  x0 = obs_s[-1]
    for t in 0..13:
        gates = x @ W_ih.T + h @ W_hh.T + (b_ih + b_hh)      # [B, 4H], order i,f,g,o
        c = sig(f)*c + sig(i)*tanh(g)
        h = sig(o)*tanh(c)
        x = h @ W_mlp.T + b_mlp                              # [B, 39] -> output step t

Algebraic folds:
  * t>=1: x_t = W_mlp h_{t-1} + b_mlp, so
        gates_t = W_combo h_{t-1} + b_combo,
        W_combo = W_ih W_mlp + W_hh,  b_combo = b_ih + b_hh + W_ih b_mlp.
  * t=0: gates_0 = (W_hh W_fc) latent + xt,
        xt = x0 W_ih.T + b_ih + b_hh + W_hh b_fc   (precomputed on host).
  * tanh(g) = 2*sigmoid(2g) - 1, with the 2x folded into the g-gate rows of
    W_combo / whf / xt on the host.  All four gates then share ONE sigmoid
    ACTIVATE per chunk (ScalarE is the roofline: 1 elem/lane/cycle with a
    ~190-cycle per-instruction overhead, so merging 4 gate activations into
    one [128, 2048] instruction is the main win).
  * x_t itself is never computed on device: the kernel emits h_t (f16) and
    the host applies W_mlp/b_mlp during output assembly.

Device layout: batch data-parallel over 8 cores (16384 each); per core
NCH=8 chunks of GROUPS=4 batch groups x C=512 columns. Activations live as
[128 partitions = 4 groups x 32 dims, C cols]. Gate matmuls use block-diag
f16 stationary weights; the 4 gates land in 4 PSUM bank-slices of one
[128, 2048] f32 tile (double-buffered = all 8 banks), with per-gate bias
accumulated via an identity-stationary matmul of a broadcast bias tile.

Per (t, chunk) engine budget: ACT does sigma([128,2048]) + its half of a
pairwise tanh(c) ([128,1024] per 2 chunks) -- ~2.4us, the wall.  DVE does
the f16 cell arithmetic (w=2g^-1 via tensor_scalar 4x mode; 3 tensor_tensor
2x ops) + h-mult.  PE does 8 FD=512 matmuls.  Output DMA is f16
[T, NCH, 128, C] h-state; mlp + upcast + bias on host in assemble_output.
"""

import numpy as np
from contextlib import ExitStack

import concourse.bass as bass
import concourse.bacc as bacc
import concourse.tile as tile
from concourse import mybir
from concourse.bass_utils import run_bass_kernel_spmd

POSE, H, LATD = 39, 32, 16
B_TOTAL, T = 131072, 14
NCORES = 8
BS = B_TOTAL // NCORES          # 16384 batch per core
NCH = 8                         # chunks per core
GROUPS = 4                      # batch groups stacked on partitions
C = BS // (NCH * GROUPS)        # 512 columns per group per chunk
# slice order inside the merged gate tile (f first so t0's i,g2,o are
# contiguous); value = PyTorch gate row block (i=0, f=1, g=2, o=3)
SLICE_PT = (1, 0, 2, 3)         # slice s -> pytorch gate index
T0_SLICES = (1, 2, 3)           # i, g2, o (f skipped at t=0: c0 = 0)
# const-pack column offsets (f16):
#   wg    4 x [128,128] block-diag W_combo.T per gate slice (g2 rows x2)
#   ident     [128,128] identity
#   whf   3 x [64,128]  block-diag (W_hh[g] W_fc).T for t0 slices (i,g2,o)
#   bbc       [128,4*C] per-gate bias broadcast tile (b_combo, g2 x2)
OW_G, OW_ID, OW_HF, OW_B = 0, 512, 640, 1024
CPACK_COLS = OW_B + 4 * C       # 3072

F32 = mybir.dt.float32
F16 = mybir.dt.float16
SIG = mybir.ActivationFunctionType.Sigmoid
TANH = mybir.ActivationFunctionType.Tanh
MULT = mybir.AluOpType.mult
ADD = mybir.AluOpType.add
SUB = mybir.AluOpType.subtract


def _build_body(ctx, tc, io):
    nc = tc.nc

    consts = ctx.enter_context(tc.tile_pool(name="consts", bufs=1))
    xin = ctx.enter_context(tc.tile_pool(name="xin", bufs=1))
    state = ctx.enter_context(tc.tile_pool(name="state", bufs=1))
    gpool = ctx.enter_context(tc.tile_pool(name="gpool", bufs=6))
    tmps = ctx.enter_context(tc.tile_pool(name="tmps", bufs=4))
    psg = ctx.enter_context(tc.tile_pool(name="psg", bufs=2, space="PSUM"))

    # ---- constants + t0 inputs to SBUF.  DMAs are split and ordered so
    # chunk 0's critical path (weights, lat, xt[k=0]) lands first; the bias
    # broadcast (first needed at t=1) goes last. ----
    cp = consts.tile([128, CPACK_COLS], F16, tag="cpack", name="cpack")
    nc.sync.dma_start(out=cp[:, 0:OW_B], in_=io["cpack"][:, 0:OW_B])
    wg = [cp[:, OW_G + 128 * s : OW_G + 128 * (s + 1)] for s in range(4)]
    ident = cp[:, OW_ID : OW_ID + 128]
    whf = [cp[0:64, OW_HF + 128 * k : OW_HF + 128 * (k + 1)] for k in range(3)]
    bbc = cp[:, OW_B : OW_B + 4 * C]

    lat = xin.tile([64, NCH * C], F16, tag="lat", name="lat")
    nc.sync.dma_start(out=lat, in_=io["lat"])
    # xt is k-major so each per-chunk DMA moves 3KB-contiguous per-partition
    # runs; dispatch alternates over otherwise-idle engine queues (each
    # dma_start costs ~650ns of serialized dispatch on its queue).
    xt = xin.tile([128, NCH, 3, C], F16, tag="xt", name="xt")
    for k in range(NCH):
        eng = (nc.sync, nc.gpsimd, nc.scalar)[k % 3]
        eng.dma_start(out=xt[:, k], in_=io["xt"][:, k])
    nc.gpsimd.dma_start(out=cp[:, OW_B:], in_=io["cpack"][:, OW_B:])

    # ---- persistent state: h double-buffered by t parity; c/tct one big
    # tile each so tanh(c) can batch chunk pairs ----
    h = {
        (par, k): state.tile([128, C], F16, tag=f"h{par}_{k}", name=f"h{par}_{k}")
        for par in range(2)
        for k in range(NCH)
    }
    cbig = state.tile([128, NCH * C], F16, tag="c", name="c")
    tct = state.tile([128, NCH * C], F16, tag="tct", name="tct")

    for t in range(T):
        par, prev = t % 2, (t - 1) % 2
        gtiles = {}
        for k in range(NCH):
            ps = psg.tile([128, 4 * C], F32, tag="ps", name="ps")
            g = gpool.tile([128, 4 * C], F16, tag="g", name="g")
            if t == 0:
                # gates_0 = (W_hh W_fc) lat + xt, slices i,g2,o only
                for gi, s in enumerate(T0_SLICES):
                    sl = slice(C * s, C * (s + 1))
                    nc.tensor.matmul(
                        ps[:, sl],
                        lhsT=whf[gi],
                        rhs=lat[:, C * k : C * (k + 1)],
                        start=True,
                        stop=False,
                    )
                    nc.tensor.matmul(
                        ps[:, sl],
                        lhsT=ident,
                        rhs=xt[:, k, gi, :],
                        start=False,
                        stop=True,
                    )
                nc.scalar.activation(g[:, C : 4 * C], ps[:, C : 4 * C], SIG)
            else:
                for s in range(4):
                    sl = slice(C * s, C * (s + 1))
                    nc.tensor.matmul(
                        ps[:, sl],
                        lhsT=wg[s],
                        rhs=h[(prev, k)],
                        start=True,
                        stop=False,
                    )
                    nc.tensor.matmul(
                        ps[:, sl],
                        lhsT=ident,
                        rhs=bbc[:, sl],
                        start=False,
                        stop=True,
                    )
                nc.scalar.activation(g, ps, SIG)
            gtiles[k] = g
            fh, ih = g[:, 0:C], g[:, C : 2 * C]
            g2h, oh = g[:, 2 * C : 3 * C], g[:, 3 * C : 4 * C]
            cs = cbig[:, C * k : C * (k + 1)]
            # w = tanh(g) = 2*sigmoid(2g) - 1  (one DVE tensor_scalar, 4x)
            w = tmps.tile([128, C], F16, tag="w", name="w")
            nc.vector.tensor_scalar(w, g2h, 2.0, 1.0, MULT, SUB)
            if t == 0:
                nc.vector.tensor_tensor(cs, ih, w, MULT)
            else:
                t2 = tmps.tile([128, C], F16, tag="t2", name="t2")
                t1 = tmps.tile([128, C], F16, tag="t1", name="t1")
                nc.vector.tensor_tensor(t2, ih, w, MULT)
                nc.vector.tensor_tensor(t1, fh, cs, MULT)
                nc.vector.tensor_tensor(cs, t1, t2, ADD)
            if k % 4 == 3:
                # quad tanh(c) -> h for chunks k-3..k
                prs = slice(C * (k - 3), C * (k + 1))
                nc.scalar.activation(tct[:, prs], cbig[:, prs], TANH)
                for kk in range(k - 3, k + 1):
                    hh = h[(par, kk)]
                    nc.vector.tensor_tensor(
                        hh,
                        gtiles[kk][:, 3 * C : 4 * C],
                        tct[:, C * kk : C * (kk + 1)],
                        MULT,
                    )
                    nc.sync.dma_start(out=io["out"][t, kk], in_=hh)


_NC_CACHE = {}


def build_nc(mode="real"):
    global _NC_CACHE
    if mode in _NC_CACHE:
        return _NC_CACHE[mode]
    nc = bacc.Bacc("TRN2", target_bir_lowering=False, debug=False)
    io = {
        "lat": nc.dram_tensor("lat", [64, NCH * C], F16, kind="ExternalInput").ap(),
        "xt": nc.dram_tensor("xt", [128, NCH, 3, C], F16, kind="ExternalInput").ap(),
        "cpack": nc.dram_tensor(
            "cpack", [128, CPACK_COLS], F16, kind="ExternalInput"
        ).ap(),
        "out": nc.dram_tensor(
            "out", [T, NCH, 128, C], F16, kind="ExternalOutput"
        ).ap(),
    }
    with tile.TileContext(nc) as tc:
        with ExitStack() as ctx:
            _build_body(ctx, tc, io)
    nc.compile()
    _NC_CACHE[mode] = nc
    return nc


def prep_inputs(obs_s, latent, W_ih, W_hh, b_ih, b_hh, W_fc, b_fc, W_mlp, b_mlp):
    """Host-side weight folding + sharding. Returns per-core input maps."""
    f32, f16 = np.float32, np.float16
    W_ih = np.asarray(W_ih, f32)
    W_hh = np.asarray(W_hh, f32)
    b_ih = np.asarray(b_ih, f32)
    b_hh = np.asarray(b_hh, f32)
    W_fc = np.asarray(W_fc, f32)
    b_fc = np.asarray(b_fc, f32)
    W_mlp = np.asarray(W_mlp, f32)
    b_mlp = np.asarray(b_mlp, f32)

    W_combo = W_ih @ W_mlp + W_hh                    # [4H, H] pytorch gate order
    b_combo = b_ih + b_hh + W_ih @ b_mlp             # [4H]

    def gscale(s):                                   # x2 on the g-gate slice
        return 2.0 if SLICE_PT[s] == 2 else 1.0

    cpack = np.zeros((128, CPACK_COLS), f32)
    for s in range(4):
        pt = SLICE_PT[s]
        blk = gscale(s) * W_combo[32 * pt : 32 * (pt + 1)].T   # [H, 32]
        for j in range(GROUPS):
            cpack[
                32 * j : 32 * (j + 1),
                OW_G + 128 * s + 32 * j : OW_G + 128 * s + 32 * (j + 1),
            ] = blk
        bvec = np.tile(gscale(s) * b_combo[32 * pt : 32 * (pt + 1)], GROUPS)
        cpack[:, OW_B + C * s : OW_B + C * (s + 1)] = bvec[:, None]
    cpack[:, OW_ID : OW_ID + 128] = np.eye(128, dtype=f32)
    for gi, s in enumerate(T0_SLICES):
        pt = SLICE_PT[s]
        blk = gscale(s) * (W_hh[32 * pt : 32 * (pt + 1)] @ W_fc).T  # [16, 32]
        for j in range(GROUPS):
            cpack[
                16 * j : 16 * (j + 1),
                OW_HF + 128 * gi + 32 * j : OW_HF + 128 * gi + 32 * (j + 1),
            ] = blk

    # xt = x0 W_ih.T + b_ih + b_hh + W_hh b_fc  (t0 gate constant), [B, 4H]
    x0 = np.asarray(obs_s[-1], f32)                       # [B, 39]
    xt_full = x0 @ W_ih.T + (b_ih + b_hh + W_hh @ b_fc)   # [B, 128] pt order
    latT = np.ascontiguousarray(np.asarray(latent, f32).T).astype(f16)  # [16, B]

    common = {"cpack": cpack.astype(f16)}
    in_maps = []
    for core in range(NCORES):
        base = core * BS
        lp = np.empty((64, NCH * C), f16)
        xp = np.empty((128, 3 * NCH * C), f16)
        for j in range(GROUPS):
            s0 = base + j * NCH * C
            lp[16 * j : 16 * (j + 1), :] = latT[:, s0 : s0 + NCH * C]
            for gi, s in enumerate(T0_SLICES):
                pt = SLICE_PT[s]
                xp[32 * j : 32 * (j + 1), NCH * C * gi : NCH * C * (gi + 1)] = (
                    gscale(s) * xt_full[s0 : s0 + NCH * C, 32 * pt : 32 * (pt + 1)].T
                )
        m = dict(common)
        m["lat"] = lp
        m["xt"] = np.ascontiguousarray(
            xp.reshape(128, 3, NCH, C).transpose(0, 2, 1, 3)
        )
        in_maps.append(m)
    return in_maps


def assemble_output(per_core_out, W_mlp, b_mlp):
    """per_core_out: list of [T, NCH, 128, C] f16 h-states -> [T, B, 39] f32.

    The device only emits h_t; the mlp head (x = h @ W_mlp.T + b_mlp) runs
    here in f32.
    """
    W_mlp = np.asarray(W_mlp, np.float32)
    b_mlp = np.asarray(b_mlp, np.float32)
    preds = np.empty((T, B_TOTAL, POSE), np.float32)
    for core in range(NCORES):
        arr = np.asarray(per_core_out[core], np.float32)
        # [T, NCH, 4*32, C] -> partition p = 32j + d holds (group j, hdim d),
        # batch b = j*NCH*C + k*C + col
        hseq = (
            arr.reshape(T, NCH, GROUPS, H, C)
            .transpose(0, 2, 1, 4, 3)
            .reshape(T, BS, H)
        )
        preds[:, core * BS : (core + 1) * BS] = hseq @ W_mlp.T + b_mlp
    return preds


def kernel(obs_s, latent, W_ih, W_hh, b_ih, b_hh, W_fc, b_fc, W_mlp, b_mlp, pred_len):
    assert int(pred_len) == T, f"kernel hardcodes pred_len={T}, got {pred_len}"
    in_maps = prep_inputs(
        obs_s, latent, W_ih, W_hh, b_ih, b_hh, W_fc, b_fc, W_mlp, b_mlp
    )
    nc = build_nc()
    res = run_bass_kernel_spmd(nc, in_maps, core_ids=list(range(NCORES)))
    return assemble_output(
        [res.results[c]["out"] for c in range(NCORES)], W_mlp, b_mlp
    )
